# revision 2
# baseline (speedup 1.0000x reference)
"""Trainium2 Bass kernel for DirectGridVolume (trilinear grid sample + SH decode).

Strategy:
  - Data-parallel over N=2M points across 8 NeuronCores.
  - Grid replicated per core, host-pre-expanded to "corner blocks": for each
    base voxel (z0,y0,x0) a 512B fp16 block holding all 8 trilinear corners
    x 28 channels (+pad to 256 elems). One dma_gather descriptor per point.
  - dma_gather uses int16 indices -> points are bucketed into z-slab windows
    of exactly 32768 blocks (2 z-slices); the window schedule is shared by
    all cores (SPMD), host sorts/pads points per core to fit it.
  - On-chip: per-tile (2048 points) compute voxel coords + trilinear weights
    (DVE/ACT), build the wrapped+replicated int16 index tile via a PE
    repeated-transpose matmul, gather, weighted-reduce 8 corners, SH basis,
    sigmoid -> density + rgb.
"""

import os
import numpy as np

P = 128
K = 16
TILE = P * K          # 2048 points per tile / per dma_gather
R = 128
C = 28
E = 256               # fp16 elements per corner-block (512 bytes)
NCORES = 8
N_FULL = 2_000_000

SCALE = float(np.float32(127.0 / 3.0))   # fx = x*SCALE + 63.5
BIAS_M05 = 63.0                          # bias for (fx - 0.5)

C0 = 0.28209479177387814
C1 = 0.4886025119029199
C2_0 = 1.0925484305920792
C2_1 = -1.0925484305920792
C2_2 = 0.31539156525252005
C2_3 = -1.0925484305920792
C2_4 = 0.5462742152960396


# ---------------------------------------------------------------- host prep

def _build_expanded_grid(density_grid, sh_grid):
    vol = np.concatenate(
        [np.asarray(density_grid, np.float32), np.asarray(sh_grid, np.float32)],
        axis=1)[0]                                   # (28, D, H, W)
    volT = np.ascontiguousarray(vol.transpose(1, 2, 3, 0))   # (z, y, x, ch)
    pad = np.pad(volT, ((0, 1), (0, 1), (0, 1), (0, 0)), mode="edge")
    Eg = np.zeros((R * R * R, E), np.float16)
    Ev = Eg[:, :8 * C].reshape(R, R, R, 8, C)
    g = 0
    for dz in (0, 1):
        for dy in (0, 1):
            for dx in (0, 1):
                Ev[:, :, :, g, :] = pad[dz:dz + R, dy:dy + R, dx:dx + R, :]
                g += 1
    return Eg


def _z0_of(xyz):
    """Base z voxel exactly as the kernel computes it (f32 affine + RNE)."""
    fz = xyz[:, 2].astype(np.float32) * np.float32(SCALE) + np.float32(BIAS_M05)
    z0 = np.rint(fz).astype(np.int32)        # RNE, matches DVE convert
    return np.clip(z0, 0, 126)


def _plan(xyz_cores):
    """Build a shared window schedule + per-core slot assignment.

    Returns (schedule, slots): schedule[t] = zbase (even) of tile t;
    slots[c] = int64 array [T_n, TILE] of original point ids (-1 = pad).
    Slot j within a tile maps to (partition j%128, block j//128).
    """
    ncores = len(xyz_cores)
    win_of = [None] * ncores
    counts = np.zeros((ncores, 64), np.int64)
    for c, xyz in enumerate(xyz_cores):
        w = _z0_of(xyz) >> 1
        win_of[c] = w
        counts[c] = np.bincount(w, minlength=64)
    maxc = counts.max(axis=0)
    tiles_per_win = np.ceil(maxc / TILE).astype(np.int64)
    schedule = []
    for w in range(64):
        schedule.extend([2 * w] * int(tiles_per_win[w]))
    T_n = len(schedule)
    slots = []
    for c in range(ncores):
        w = win_of[c]
        order = np.argsort(w, kind="stable")
        arr = np.full((T_n, TILE), -1, np.int64)
        pos = 0
        t = 0
        for wi in range(64):
            nw = int(counts[c, wi])
            ntw = int(tiles_per_win[wi])
            pts = order[pos:pos + nw]
            pos += nw
            for k in range(ntw):
                chunk = pts[k * TILE:(k + 1) * TILE]
                arr[t, :len(chunk)] = chunk
                t += 1
        slots.append(arr)
    return schedule, slots


def _pack_points(xyz, vd, slot_ids, schedule):
    """Build device-side packed point arrays for one core."""
    T_n = len(schedule)
    n = T_n * TILE
    xyz_f = np.zeros((n, 3), np.float32)
    vd_f = np.zeros((n, 3), np.float32)
    vd_f[:, 2] = 1.0
    flat = slot_ids.reshape(-1)
    valid = flat >= 0
    xyz_f[valid] = xyz[flat[valid]]
    vd_f[valid] = vd[flat[valid]]
    # synthetic pads: land exactly on z0 == zbase of their tile, x=y=0
    zb = np.repeat(np.asarray(schedule, np.float32), TILE)
    pad_z = ((zb + 0.5) - 63.5) / np.float32(SCALE)
    xyz_f[~valid, 2] = pad_z[~valid]

    pts = xyz_f.reshape(T_n, K, P, 3)      # slot j = c*128+p -> [t, c, p, :]
    xyzA = np.ascontiguousarray(pts.transpose(0, 2, 1, 3)).reshape(T_n, P, 3 * K)
    vdp = vd_f.reshape(T_n, K, P, 3)
    vdA = np.ascontiguousarray(vdp.transpose(0, 2, 1, 3)).reshape(T_n, P, 3 * K)
    # B layout (p-major): xyzB[t, p, c*3+d] = point (p*16+c)
    ptsB = xyz_f.reshape(T_n, P, K, 3)
    xyzB = np.ascontiguousarray(ptsB).reshape(T_n, P, 3 * K)
    return xyzA, vdA, xyzB


# ---------------------------------------------------------------- bass build

def _build_kernel(T_n, schedule):
    import concourse.bacc as bacc
    import concourse.mybir as mybir
    import concourse.tile as tile
    from concourse.masks import make_identity

    f32 = mybir.dt.float32
    f16 = mybir.dt.float16
    i32 = mybir.dt.int32
    i16 = mybir.dt.int16
    Alu = mybir.AluOpType
    Act = mybir.ActivationFunctionType

    nc = bacc.Bacc("TRN2", target_bir_lowering=False, debug=False)
    xyzA_d = nc.dram_tensor("xyzA", [T_n, P, 3 * K], f32, kind="ExternalInput")
    vdA_d = nc.dram_tensor("vdA", [T_n, P, 3 * K], f32, kind="ExternalInput")
    xyzB_d = nc.dram_tensor("xyzB", [T_n, P, 3 * K], f32, kind="ExternalInput")
    grid_d = nc.dram_tensor("grid", [R * R * R, E], f16, kind="ExternalInput")
    dens_d = nc.dram_tensor("dens", [T_n, P, K], f32, kind="ExternalOutput")
    rgb_d = nc.dram_tensor("rgb", [T_n, P, 3 * K], f32, kind="ExternalOutput")

    def a3(ap):
        return ap.rearrange("p (k d) -> p k d", d=3)

    with tile.TileContext(nc) as tc:
        with tc.tile_pool(name="const", bufs=1) as cpool, \
             tc.tile_pool(name="sb", bufs=3) as pool, \
             tc.tile_pool(name="vals", bufs=3) as vpool, \
             tc.tile_pool(name="ps", bufs=2, space="PSUM") as psp:
            ident = cpool.tile([P, P], f32)
            make_identity(nc, ident[:])

            for t in range(T_n):
                zb = schedule[t]
                # ---- load points
                xyzA = pool.tile([P, 3 * K], f32, tag="xyzA")
                nc.sync.dma_start(xyzA[:], xyzA_d[t])
                vdA = pool.tile([P, 3 * K], f32, tag="vdA")
                nc.sync.dma_start(vdA[:], vdA_d[t])
                xyzB = pool.tile([P, 3 * K], f32, tag="xyzB")
                nc.sync.dma_start(xyzB[:], xyzB_d[t])

                # ---- A chain: voxel coords + lerp weights (c-major slots)
                fA = pool.tile([P, 3 * K], f32, tag="fA")     # f - 0.5 (z: -zb)
                nc.scalar.activation(a3(fA[:])[:, :, 0:2], a3(xyzA[:])[:, :, 0:2],
                                     Act.Copy, bias=BIAS_M05, scale=SCALE)
                nc.scalar.activation(a3(fA[:])[:, :, 2], a3(xyzA[:])[:, :, 2],
                                     Act.Copy, bias=float(BIAS_M05 - zb), scale=SCALE)
                iA = pool.tile([P, 3 * K], i32, tag="iA")
                nc.vector.tensor_copy(iA[:], fA[:])           # RNE -> floor(f)
                iAf = pool.tile([P, 3 * K], f32, tag="iAf")
                nc.vector.tensor_copy(iAf[:], iA[:])
                nc.vector.tensor_scalar(a3(iAf[:])[:, :, 0:2], a3(iAf[:])[:, :, 0:2],
                                        0.0, 126.0, op0=Alu.max, op1=Alu.min)
                nc.vector.tensor_scalar(a3(iAf[:])[:, :, 2], a3(iAf[:])[:, :, 2],
                                        0.0, 1.0, op0=Alu.max, op1=Alu.min)
                # w01: [:, 0:48] = w0, [:, 48:96] = w1
                w01 = pool.tile([P, 6 * K], f32, tag="w01")
                w1v = w01[:, 3 * K:6 * K]
                nc.vector.scalar_tensor_tensor(w1v, fA[:], 0.5, iAf[:],
                                               op0=Alu.add, op1=Alu.subtract)
                nc.scalar.activation(w01[:, 0:3 * K], w1v, Act.Copy,
                                     bias=1.0, scale=-1.0)
                # corner weights w8[p, k*8 + (dz*4+dy*2+dx)]
                wp = w01[:].rearrange("p (o k d) -> p k d o", o=2, d=3)
                in_z = wp[:, :, 2, :].unsqueeze(3).broadcast_to([P, K, 2, 2])
                in_y = wp[:, :, 1, :].unsqueeze(2).broadcast_to([P, K, 2, 2])
                wzy = pool.tile([P, 4 * K], f32, tag="wzy")
                nc.vector.tensor_tensor(
                    wzy[:].rearrange("p (k a b) -> p k a b", a=2, b=2),
                    in_z, in_y, op=Alu.mult)
                in_zy = (wzy[:].rearrange("p (k z) -> p k z", z=4)
                         .unsqueeze(3).broadcast_to([P, K, 4, 2]))
                in_x = wp[:, :, 0, :].unsqueeze(2).broadcast_to([P, K, 4, 2])
                w8 = pool.tile([P, 8 * K], f32, tag="w8")
                nc.vector.tensor_tensor(
                    w8[:].rearrange("p (k g x) -> p k g x", g=4, x=2),
                    in_zy, in_x, op=Alu.mult)

                # ---- B chain: block index per point (p-major slots)
                fB = pool.tile([P, 3 * K], f32, tag="fB")
                nc.scalar.activation(a3(fB[:])[:, :, 0:2], a3(xyzB[:])[:, :, 0:2],
                                     Act.Copy, bias=BIAS_M05, scale=SCALE)
                nc.scalar.activation(a3(fB[:])[:, :, 2], a3(xyzB[:])[:, :, 2],
                                     Act.Copy, bias=float(BIAS_M05 - zb), scale=SCALE)
                iB = pool.tile([P, 3 * K], i32, tag="iB")
                nc.vector.tensor_copy(iB[:], fB[:])
                iBf = pool.tile([P, 3 * K], f32, tag="iBf")
                nc.vector.tensor_copy(iBf[:], iB[:])
                nc.vector.tensor_scalar(a3(iBf[:])[:, :, 0:2], a3(iBf[:])[:, :, 0:2],
                                        0.0, 126.0, op0=Alu.max, op1=Alu.min)
                nc.vector.tensor_scalar(a3(iBf[:])[:, :, 2], a3(iBf[:])[:, :, 2],
                                        0.0, 1.0, op0=Alu.max, op1=Alu.min)
                t1 = pool.tile([P, K], f32, tag="t1")
                nc.vector.scalar_tensor_tensor(t1[:], a3(iBf[:])[:, :, 2], 128.0,
                                               a3(iBf[:])[:, :, 1],
                                               op0=Alu.mult, op1=Alu.add)
                idxB = pool.tile([P, K], f32, tag="idxB")
                nc.vector.scalar_tensor_tensor(idxB[:], t1[:], 128.0,
                                               a3(iBf[:])[:, :, 0],
                                               op0=Alu.mult, op1=Alu.add)
                # wrapped + replicated idx tile via PE: W[q*16+c, p] = idxB[p, c]
                wrep = pool.tile([P, P], f32, tag="wrep")
                nc.vector.tensor_copy(
                    wrep[:].rearrange("p (q c) -> p q c", c=K),
                    idxB[:].unsqueeze(1).broadcast_to([P, 8, K]))
                ps = psp.tile([P, P], f32, tag="ps", space="PSUM")
                nc.tensor.matmul(ps[:], wrep[:], ident[:], start=True, stop=True)
                W128 = pool.tile([P, P], i16, tag="W128")
                nc.vector.tensor_copy(W128[:], ps[:])

                # ---- gather 2048 corner-blocks (512B fp16 each)
                vals = vpool.tile([P, K * E], f16, tag="vals")
                win = grid_d[zb * 16384: zb * 16384 + 32768, :]
                nc.gpsimd.dma_gather(
                    vals[:].rearrange("p (k e) -> p k e", e=E),
                    win, W128[:], TILE, TILE, E, single_packet=False)

                # ---- weighted corner reduce -> feats [P, K*28]
                valsf = vpool.tile([P, K * 8 * C], f32, tag="valsf")
                v4 = (vals[:].rearrange("p (k e) -> p k e", e=E)[:, :, 0:8 * C]
                      .rearrange("p k (g c) -> p k g c", c=C))
                w8b = (w8[:].rearrange("p (k g) -> p k g", g=8)
                       .unsqueeze(3).broadcast_to([P, K, 8, C]))
                nc.vector.tensor_tensor(
                    valsf[:].rearrange("p (k g c) -> p k g c", g=8, c=C),
                    v4, w8b, op=Alu.mult)
                vf3 = valsf[:].rearrange("p (k e) -> p k e", e=8 * C)
                nc.vector.tensor_tensor(vf3[:, :, 0:4 * C], vf3[:, :, 0:4 * C],
                                        vf3[:, :, 4 * C:8 * C], op=Alu.add)
                nc.vector.tensor_tensor(vf3[:, :, 0:2 * C], vf3[:, :, 0:2 * C],
                                        vf3[:, :, 2 * C:4 * C], op=Alu.add)
                feats = pool.tile([P, K * C], f32, tag="feats")
                feats3 = feats[:].rearrange("p (k c) -> p k c", c=C)
                nc.vector.tensor_tensor(feats3, vf3[:, :, 0:C], vf3[:, :, C:2 * C],
                                        op=Alu.add)

                # ---- density = relu(feats[:, :, 0])
                dens = pool.tile([P, K], f32, tag="dens")
                nc.scalar.activation(dens[:], feats3[:, :, 0], Act.Relu)
                nc.sync.dma_start(dens_d[t], dens[:])

                # ---- SH basis [P, K*9] (slot k*9+b)
                vd3 = a3(vdA[:])
                x_v, y_v, z_v = vd3[:, :, 0], vd3[:, :, 1], vd3[:, :, 2]
                sq = pool.tile([P, 3 * K], f32, tag="sq")
                nc.vector.tensor_tensor(sq[:], vdA[:], vdA[:], op=Alu.mult)
                sq3 = a3(sq[:])
                xx, yy, zz = sq3[:, :, 0], sq3[:, :, 1], sq3[:, :, 2]
                sxy = pool.tile([P, K], f32, tag="sxy")
                nc.vector.tensor_tensor(sxy[:], xx, yy, op=Alu.add)
                bas = pool.tile([P, 9 * K], f32, tag="bas")
                b9 = bas[:].rearrange("p (k b) -> p k b", b=9)
                nc.scalar.activation(b9[:, :, 0], x_v, Act.Copy, bias=C0, scale=0.0)
                nc.scalar.activation(b9[:, :, 1], y_v, Act.Copy, scale=-C1)
                nc.scalar.activation(b9[:, :, 2], z_v, Act.Copy, scale=C1)
                nc.scalar.activation(b9[:, :, 3], x_v, Act.Copy, scale=-C1)
                nc.vector.scalar_tensor_tensor(b9[:, :, 4], x_v, C2_0, y_v,
                                               op0=Alu.mult, op1=Alu.mult)
                nc.vector.scalar_tensor_tensor(b9[:, :, 5], y_v, C2_1, z_v,
                                               op0=Alu.mult, op1=Alu.mult)
                t6 = pool.tile([P, K], f32, tag="t6")
                nc.vector.scalar_tensor_tensor(t6[:], zz, 2.0, sxy[:],
                                               op0=Alu.mult, op1=Alu.subtract)
                nc.scalar.activation(b9[:, :, 6], t6[:], Act.Copy, scale=C2_2)
                nc.vector.scalar_tensor_tensor(b9[:, :, 7], x_v, C2_3, z_v,
                                               op0=Alu.mult, op1=Alu.mult)
                t8 = pool.tile([P, K], f32, tag="t8")
                nc.vector.tensor_tensor(t8[:], xx, yy, op=Alu.subtract)
                nc.scalar.activation(b9[:, :, 8], t8[:], Act.Copy, scale=C2_4)

                # ---- rgb = sigmoid(sum_b sh[k,j,b]*basis[k,b])
                prod = pool.tile([P, K * 27], f32, tag="prod")
                sh_ap = (feats3[:, :, 1:C]
                         .rearrange("p k (j b) -> p k j b", b=9))
                bas_ap = b9.unsqueeze(2).broadcast_to([P, K, 3, 9])
                nc.vector.tensor_tensor(
                    prod[:].rearrange("p (k j b) -> p k j b", j=3, b=9),
                    sh_ap, bas_ap, op=Alu.mult)
                rgbr = pool.tile([P, 3 * K], f32, tag="rgbr")
                nc.vector.tensor_reduce(
                    rgbr[:].rearrange("p (k j) -> p k j", j=3),
                    prod[:].rearrange("p (kj b) -> p kj b", b=9),
                    axis=mybir.AxisListType.X, op=Alu.add)
                rgbs = pool.tile([P, 3 * K], f32, tag="rgbs")
                nc.scalar.activation(rgbs[:], rgbr[:], Act.Sigmoid)
                nc.sync.dma_start(rgb_d[t], rgbs[:])
    nc.compile()
    return nc


# ---------------------------------------------------------------- entry point

def kernel(xyz, view_dirs, density_grid, sh_grid):
    import sys
    if '/root/problem/work' not in sys.path:
        sys.path.insert(0, '/root/problem/work')
    try:
        import ntff_shim
        ntff_shim.install()
    except Exception:
        pass
    from concourse.bass_utils import run_bass_kernel_spmd

    xyz = np.asarray(xyz, np.float32)
    view_dirs = np.asarray(view_dirs, np.float32)
    N = xyz.shape[0]
    ncores = NCORES
    per = (N + ncores - 1) // ncores
    xyz_cores = [xyz[c * per:(c + 1) * per] for c in range(ncores)]
    vd_cores = [view_dirs[c * per:(c + 1) * per] for c in range(ncores)]

    grid = _build_expanded_grid(density_grid, sh_grid)
    schedule, slots = _plan(xyz_cores)
    T_n = len(schedule)

    in_maps = []
    for ci in range(ncores):
        xyzA, vdA, xyzB = _pack_points(xyz_cores[ci], vd_cores[ci],
                                       slots[ci], schedule)
        in_maps.append({"xyzA": xyzA, "vdA": vdA, "xyzB": xyzB, "grid": grid})

    nc = _build_kernel(T_n, schedule)
    trace = bool(int(os.environ.get("KERNEL_TRACE", "0")))
    res = run_bass_kernel_spmd(
        nc, in_maps, core_ids=list(range(ncores)), trace=trace,
        trace_cores=[0] if trace else None)
    kernel.last_exec_time_ns = res.exec_time_ns

    density = np.zeros((N,), np.float32)
    rgb = np.zeros((N, 3), np.float32)
    for ci in range(ncores):
        r = res.results[ci]
        dd = r["dens"]                    # [T_n, P, K]  (p, c) = slot c*128+p
        rr = r["rgb"].reshape(T_n, P, K, 3)
        ids = slots[ci]                   # [T_n, TILE], slot j = c*128+p
        ids_pc = ids.reshape(T_n, K, P).transpose(0, 2, 1)   # [T_n, P, K]
        valid = ids_pc >= 0
        base = ci * per
        density[base + ids_pc[valid]] = dd[valid]
        rgb[base + ids_pc[valid]] = rr[valid]
    return density, rgb


kernel.last_exec_time_ns = None


# revision 4
# speedup vs baseline: 1.8988x; 1.8988x over previous
"""Trainium2 Bass kernel for DirectGridVolume (trilinear grid sample + SH decode).

Strategy:
  - Data-parallel over N=2M points across 8 NeuronCores.
  - Grid replicated per core, host-pre-expanded to "corner blocks": for each
    base voxel (z0,y0,x0) a 512B fp16 block holding all 8 trilinear corners
    x 28 channels (+pad to 256 elems). One dma_gather descriptor per point.
  - dma_gather uses int16 indices -> points are bucketed into z-slab windows
    of exactly 32768 blocks (2 z-slices); the window schedule is shared by
    all cores (SPMD), host sorts/pads points per core to fit it.
  - On-chip per tile (2048 points): voxel coords + trilinear weights on
    ACT/DVE (planar layouts, contiguous APs), wrapped+replicated int16 index
    tile via a PE repeated-transpose matmul, 4 sub-gathers on 4 SWDGE queues
    (parallel Q7 descriptor generation), weighted 8-corner reduce, SH basis,
    sigmoid -> density + rgb.
"""

import os
import numpy as np

P = 128
K = 16
TILE = P * K          # 2048 points per tile
R = 128
C = 28
E = 256               # fp16 elements per corner-block (512 bytes)
NCORES = 8
NQ = 4                # SWDGE queues; sub-gathers per tile
SUB = TILE // NQ      # 512 indices per sub-gather

SCALE = float(np.float32(127.0 / 3.0))   # fx = x*SCALE + 63.5
BIAS_M05 = 63.0                          # bias for (fx - 0.5)

C0 = 0.28209479177387814
C1 = 0.4886025119029199
C2_0 = 1.0925484305920792
C2_1 = -1.0925484305920792
C2_2 = 0.31539156525252005
C2_3 = -1.0925484305920792
C2_4 = 0.5462742152960396


# ---------------------------------------------------------------- host prep

def _build_expanded_grid(density_grid, sh_grid):
    vol = np.concatenate(
        [np.asarray(density_grid, np.float32), np.asarray(sh_grid, np.float32)],
        axis=1)[0]                                   # (28, D, H, W)
    volT = np.ascontiguousarray(vol.transpose(1, 2, 3, 0))   # (z, y, x, ch)
    pad = np.pad(volT, ((0, 1), (0, 1), (0, 1), (0, 0)), mode="edge")
    Eg = np.zeros((R * R * R, E), np.float16)
    Ev = Eg[:, :8 * C].reshape(R, R, R, 8, C)
    g = 0
    for dz in (0, 1):
        for dy in (0, 1):
            for dx in (0, 1):
                Ev[:, :, :, g, :] = pad[dz:dz + R, dy:dy + R, dx:dx + R, :]
                g += 1
    return Eg


def _z0_of(xyz):
    """Base z voxel exactly as the kernel computes it (f32 affine + RNE)."""
    fz = xyz[:, 2].astype(np.float32) * np.float32(SCALE) + np.float32(BIAS_M05)
    z0 = np.rint(fz).astype(np.int32)        # RNE, matches DVE convert
    return np.clip(z0, 0, 126)


def _plan(xyz_cores):
    """Shared window schedule + per-core slot assignment.

    schedule[t] = zbase (even) of tile t; slots[c] = int64 [T_n, TILE] of
    original point ids (-1 = pad). Slot j maps to (partition j%128, j//128).
    """
    ncores = len(xyz_cores)
    win_of = [None] * ncores
    counts = np.zeros((ncores, 64), np.int64)
    for c, xyz in enumerate(xyz_cores):
        w = _z0_of(xyz) >> 1
        win_of[c] = w
        counts[c] = np.bincount(w, minlength=64)
    maxc = counts.max(axis=0)
    tiles_per_win = np.ceil(maxc / TILE).astype(np.int64)
    schedule = []
    for w in range(64):
        schedule.extend([2 * w] * int(tiles_per_win[w]))
    T_n = len(schedule)
    slots = []
    for c in range(ncores):
        w = win_of[c]
        order = np.argsort(w, kind="stable")
        arr = np.full((T_n, TILE), -1, np.int64)
        pos = 0
        t = 0
        for wi in range(64):
            nw = int(counts[c, wi])
            ntw = int(tiles_per_win[wi])
            pts = order[pos:pos + nw]
            pos += nw
            for k in range(ntw):
                chunk = pts[k * TILE:(k + 1) * TILE]
                arr[t, :len(chunk)] = chunk
                t += 1
        slots.append(arr)
    return schedule, slots


def _pack_points(xyz, vd, slot_ids, schedule):
    """Device-side packed point arrays for one core (planar layouts)."""
    T_n = len(schedule)
    n = T_n * TILE
    xyz_f = np.zeros((n, 3), np.float32)
    vd_f = np.zeros((n, 3), np.float32)
    vd_f[:, 2] = 1.0
    flat = slot_ids.reshape(-1)
    valid = flat >= 0
    xyz_f[valid] = xyz[flat[valid]]
    vd_f[valid] = vd[flat[valid]]
    zb = np.repeat(np.asarray(schedule, np.float32), TILE)
    pad_z = ((zb + 0.5) - 63.5) / np.float32(SCALE)
    xyz_f[~valid, 2] = pad_z[~valid]

    # A half (c-major): (p, d, k) = xyz[slot k*128+p, d]
    ptsA = xyz_f.reshape(T_n, K, P, 3).transpose(0, 2, 3, 1)   # [t, p, d, k]
    # B half (p-major): (p, d, k) = xyz[slot p*16+k, d]
    ptsB = xyz_f.reshape(T_n, P, K, 3).transpose(0, 1, 3, 2)   # [t, p, d, k]
    xyzAB = np.concatenate(
        [ptsA.reshape(T_n, P, 48), ptsB.reshape(T_n, P, 48)], axis=2)
    xyzAB = np.ascontiguousarray(xyzAB)
    vdA = np.ascontiguousarray(
        vd_f.reshape(T_n, K, P, 3).transpose(0, 2, 3, 1)).reshape(T_n, P, 48)
    return xyzAB, vdA


# ---------------------------------------------------------------- bass build

def _build_kernel(T_n, schedule):
    import concourse.bacc as bacc
    import concourse.mybir as mybir
    import concourse.tile as tile
    from concourse.masks import make_identity

    f32 = mybir.dt.float32
    f16 = mybir.dt.float16
    i32 = mybir.dt.int32
    i16 = mybir.dt.int16
    Alu = mybir.AluOpType
    Act = mybir.ActivationFunctionType

    nc = bacc.Bacc("TRN2", target_bir_lowering=False, debug=False,
                   num_swdge_queues=NQ)
    xyzAB_d = nc.dram_tensor("xyzAB", [T_n, P, 96], f32, kind="ExternalInput")
    vdA_d = nc.dram_tensor("vdA", [T_n, P, 48], f32, kind="ExternalInput")
    grid_d = nc.dram_tensor("grid", [R * R * R, E], f16, kind="ExternalInput")
    dens_d = nc.dram_tensor("dens", [T_n, P, K], f32, kind="ExternalOutput")
    rgb_d = nc.dram_tensor("rgb", [T_n, P, 3 * K], f32, kind="ExternalOutput")

    with tile.TileContext(nc) as tc:
        with tc.tile_pool(name="const", bufs=1) as cpool, \
             tc.tile_pool(name="sb", bufs=3) as pool, \
             tc.tile_pool(name="vals", bufs=3) as vpool, \
             tc.tile_pool(name="ps", bufs=2, space="PSUM") as psp:
            ident = cpool.tile([P, P], f32)
            make_identity(nc, ident[:])

            for t in range(T_n):
                zb = schedule[t]
                xyzAB = pool.tile([P, 96], f32, tag="xyzAB")
                nc.sync.dma_start(xyzAB[:], xyzAB_d[t])
                vdA = pool.tile([P, 48], f32, tag="vdA")
                nc.sync.dma_start(vdA[:], vdA_d[t])

                # halves: col = h*48 + d*16 + k  (planar)
                def hd(ap):
                    return ap.rearrange("p (h d k) -> p h d k", h=2, d=3)

                # ---- affine to (f - 0.5) voxel coords; z shifted by -zb
                fAB = pool.tile([P, 96], f32, tag="fAB")
                nc.scalar.activation(hd(fAB[:])[:, :, 0:2, :],
                                     hd(xyzAB[:])[:, :, 0:2, :],
                                     Act.Copy, bias=BIAS_M05, scale=SCALE)
                nc.scalar.activation(hd(fAB[:])[:, :, 2, :],
                                     hd(xyzAB[:])[:, :, 2, :],
                                     Act.Copy, bias=float(BIAS_M05 - zb), scale=SCALE)
                # ---- floor via RNE(f - 0.5), clamp
                iAB = pool.tile([P, 96], i32, tag="iAB")
                nc.vector.tensor_copy(iAB[:], fAB[:])
                fI = pool.tile([P, 96], f32, tag="fI")
                nc.vector.tensor_copy(fI[:], iAB[:])
                nc.vector.tensor_scalar(hd(fI[:])[:, :, 0:2, :],
                                        hd(fI[:])[:, :, 0:2, :],
                                        0.0, 126.0, op0=Alu.max, op1=Alu.min)
                nc.vector.tensor_scalar(hd(fI[:])[:, :, 2, :],
                                        hd(fI[:])[:, :, 2, :],
                                        0.0, 1.0, op0=Alu.max, op1=Alu.min)

                # ---- lerp weights (A half): w01 = [w0(48) | w1(48)] planar
                w01 = pool.tile([P, 96], f32, tag="w01")
                w1v = w01[:, 48:96]
                nc.vector.scalar_tensor_tensor(w1v, fAB[:, 0:48], 0.5,
                                               fI[:, 0:48],
                                               op0=Alu.add, op1=Alu.subtract)
                nc.scalar.activation(w01[:, 0:48], w1v, Act.Copy,
                                     bias=1.0, scale=-1.0)
                # corner weights, g-outer k-inner: w82[p, g*16+k], g=dz*4+dy*2+dx
                wod = w01[:].rearrange("p (o d k) -> p o d k", o=2, d=3)
                in_z = wod[:, :, 2, :].unsqueeze(2).broadcast_to([P, 2, 2, K])
                in_y = wod[:, :, 1, :].unsqueeze(1).broadcast_to([P, 2, 2, K])
                wzy = pool.tile([P, 4 * K], f32, tag="wzy")
                nc.vector.tensor_tensor(
                    wzy[:].rearrange("p (a b k) -> p a b k", a=2, b=2),
                    in_z, in_y, op=Alu.mult)
                in_zy = (wzy[:].rearrange("p (zy k) -> p zy k", zy=4)
                         .unsqueeze(2).broadcast_to([P, 4, 2, K]))
                in_x = wod[:, :, 0, :].unsqueeze(1).broadcast_to([P, 4, 2, K])
                w82 = pool.tile([P, 8 * K], f32, tag="w82")
                nc.vector.tensor_tensor(
                    w82[:].rearrange("p (zy x k) -> p zy x k", zy=4, x=2),
                    in_zy, in_x, op=Alu.mult)

                # ---- block index (B half): idx = (z'*128 + y)*128 + x
                Bx, By, Bz = fI[:, 48:64], fI[:, 64:80], fI[:, 80:96]
                t1 = pool.tile([P, K], f32, tag="t1")
                nc.vector.scalar_tensor_tensor(t1[:], Bz, 128.0, By,
                                               op0=Alu.mult, op1=Alu.add)
                idxB = pool.tile([P, K], f32, tag="idxB")
                nc.vector.scalar_tensor_tensor(idxB[:], t1[:], 128.0, Bx,
                                               op0=Alu.mult, op1=Alu.add)
                # wrapped + replicated idx tile via PE: W[q*16+c, p] = idxB[p, c]
                wrep = pool.tile([P, P], f32, tag="wrep")
                nc.vector.tensor_copy(
                    wrep[:].rearrange("p (q c) -> p q c", c=K),
                    idxB[:].unsqueeze(1).broadcast_to([P, 8, K]))
                ps = psp.tile([P, P], f32, tag="ps", space="PSUM")
                nc.tensor.matmul(ps[:], wrep[:], ident[:], start=True, stop=True)
                W128 = pool.tile([P, P], i16, tag="W128")
                nc.vector.tensor_copy(W128[:], ps[:])

                # ---- gather 2048 corner-blocks, 4 sub-gathers on 4 queues
                vals = vpool.tile([P, K * E], f16, tag="vals")
                win = grid_d[zb * 16384: zb * 16384 + 32768, :]
                v3 = vals[:].rearrange("p (k e) -> p k e", e=E)
                for q in range(NQ):
                    nsub = SUB // P          # blocks per partition per sub
                    nc.gpsimd.dma_gather(
                        v3[:, q * nsub:(q + 1) * nsub, :],
                        win, W128[:, q * (SUB // 16):(q + 1) * (SUB // 16)],
                        SUB, SUB, E, single_packet=False, queue_num=q)

                # ---- weighted corner reduce -> feats [P, K*28]
                valsf = vpool.tile([P, K * 8 * C], f32, tag="valsf")
                v4 = (vals[:].rearrange("p (k e) -> p k e", e=E)[:, :, 0:8 * C]
                      .rearrange("p k (g c) -> p k g c", c=C))
                w8b = (w82[:].rearrange("p (g k) -> p k g", g=8)
                       .unsqueeze(3).broadcast_to([P, K, 8, C]))
                nc.vector.tensor_tensor(
                    valsf[:].rearrange("p (k g c) -> p k g c", g=8, c=C),
                    v4, w8b, op=Alu.mult)
                vf3 = valsf[:].rearrange("p (k e) -> p k e", e=8 * C)
                nc.vector.tensor_tensor(vf3[:, :, 0:4 * C], vf3[:, :, 0:4 * C],
                                        vf3[:, :, 4 * C:8 * C], op=Alu.add)
                nc.vector.tensor_tensor(vf3[:, :, 0:2 * C], vf3[:, :, 0:2 * C],
                                        vf3[:, :, 2 * C:4 * C], op=Alu.add)
                feats = pool.tile([P, K * C], f32, tag="feats")
                feats3 = feats[:].rearrange("p (k c) -> p k c", c=C)
                nc.vector.tensor_tensor(feats3, vf3[:, :, 0:C], vf3[:, :, C:2 * C],
                                        op=Alu.add)

                # ---- density = relu(feats[:, :, 0])
                dens = pool.tile([P, K], f32, tag="dens")
                nc.scalar.activation(dens[:], feats3[:, :, 0], Act.Relu)
                nc.sync.dma_start(dens_d[t], dens[:])

                # ---- SH basis [P, K*9] (slot k*9+b); vd planar [x16 y16 z16]
                x_v, y_v, z_v = vdA[:, 0:16], vdA[:, 16:32], vdA[:, 32:48]
                sq = pool.tile([P, 48], f32, tag="sq")
                nc.vector.tensor_tensor(sq[:], vdA[:], vdA[:], op=Alu.mult)
                xx, yy, zz = sq[:, 0:16], sq[:, 16:32], sq[:, 32:48]
                sxy = pool.tile([P, K], f32, tag="sxy")
                nc.vector.tensor_tensor(sxy[:], xx, yy, op=Alu.add)
                bas = pool.tile([P, 9 * K], f32, tag="bas")
                b9 = bas[:].rearrange("p (k b) -> p k b", b=9)
                nc.scalar.activation(b9[:, :, 0], x_v, Act.Copy, bias=C0, scale=0.0)
                nc.scalar.activation(b9[:, :, 1], y_v, Act.Copy, scale=-C1)
                nc.scalar.activation(b9[:, :, 2], z_v, Act.Copy, scale=C1)
                nc.scalar.activation(b9[:, :, 3], x_v, Act.Copy, scale=-C1)
                nc.vector.scalar_tensor_tensor(b9[:, :, 4], x_v, C2_0, y_v,
                                               op0=Alu.mult, op1=Alu.mult)
                nc.vector.scalar_tensor_tensor(b9[:, :, 5], y_v, C2_1, z_v,
                                               op0=Alu.mult, op1=Alu.mult)
                t6 = pool.tile([P, K], f32, tag="t6")
                nc.vector.scalar_tensor_tensor(t6[:], zz, 2.0, sxy[:],
                                               op0=Alu.mult, op1=Alu.subtract)
                nc.scalar.activation(b9[:, :, 6], t6[:], Act.Copy, scale=C2_2)
                nc.vector.scalar_tensor_tensor(b9[:, :, 7], x_v, C2_3, z_v,
                                               op0=Alu.mult, op1=Alu.mult)
                t8 = pool.tile([P, K], f32, tag="t8")
                nc.vector.tensor_tensor(t8[:], xx, yy, op=Alu.subtract)
                nc.scalar.activation(b9[:, :, 8], t8[:], Act.Copy, scale=C2_4)

                # ---- rgb = sigmoid(sum_b sh[k,j,b]*basis[k,b])
                prod = pool.tile([P, K * 27], f32, tag="prod")
                sh_ap = (feats3[:, :, 1:C]
                         .rearrange("p k (j b) -> p k j b", b=9))
                bas_ap = b9.unsqueeze(2).broadcast_to([P, K, 3, 9])
                nc.vector.tensor_tensor(
                    prod[:].rearrange("p (k j b) -> p k j b", j=3, b=9),
                    sh_ap, bas_ap, op=Alu.mult)
                rgbr = pool.tile([P, 3 * K], f32, tag="rgbr")
                nc.vector.tensor_reduce(
                    rgbr[:].rearrange("p (k j) -> p k j", j=3),
                    prod[:].rearrange("p (kj b) -> p kj b", b=9),
                    axis=mybir.AxisListType.X, op=Alu.add)
                rgbs = pool.tile([P, 3 * K], f32, tag="rgbs")
                nc.scalar.activation(rgbs[:], rgbr[:], Act.Sigmoid)
                nc.sync.dma_start(rgb_d[t], rgbs[:])
    nc.compile()
    return nc


# ---------------------------------------------------------------- entry point

def kernel(xyz, view_dirs, density_grid, sh_grid):
    import sys
    if '/root/problem/work' not in sys.path:
        sys.path.insert(0, '/root/problem/work')
    try:
        import ntff_shim
        ntff_shim.install()
    except Exception:
        pass
    from concourse.bass_utils import run_bass_kernel_spmd

    xyz = np.asarray(xyz, np.float32)
    view_dirs = np.asarray(view_dirs, np.float32)
    N = xyz.shape[0]
    ncores = NCORES
    per = (N + ncores - 1) // ncores
    xyz_cores = [xyz[c * per:(c + 1) * per] for c in range(ncores)]
    vd_cores = [view_dirs[c * per:(c + 1) * per] for c in range(ncores)]

    grid = _build_expanded_grid(density_grid, sh_grid)
    schedule, slots = _plan(xyz_cores)
    T_n = len(schedule)

    in_maps = []
    for ci in range(ncores):
        xyzAB, vdA = _pack_points(xyz_cores[ci], vd_cores[ci],
                                  slots[ci], schedule)
        in_maps.append({"xyzAB": xyzAB, "vdA": vdA, "grid": grid})

    nc = _build_kernel(T_n, schedule)
    trace = bool(int(os.environ.get("KERNEL_TRACE", "0")))
    res = run_bass_kernel_spmd(
        nc, in_maps, core_ids=list(range(ncores)), trace=trace,
        trace_cores=[0] if trace else None)
    kernel.last_exec_time_ns = res.exec_time_ns

    density = np.zeros((N,), np.float32)
    rgb = np.zeros((N, 3), np.float32)
    for ci in range(ncores):
        r = res.results[ci]
        dd = r["dens"]                    # [T_n, P, K]  (p, c) = slot c*128+p
        rr = r["rgb"].reshape(T_n, P, K, 3)
        ids = slots[ci]                   # [T_n, TILE], slot j = c*128+p
        ids_pc = ids.reshape(T_n, K, P).transpose(0, 2, 1)   # [T_n, P, K]
        valid = ids_pc >= 0
        base = ci * per
        density[base + ids_pc[valid]] = dd[valid]
        rgb[base + ids_pc[valid]] = rr[valid]
    return density, rgb


kernel.last_exec_time_ns = None
